# revision 7
# baseline (speedup 1.0000x reference)
"""Trainium2 Bass kernel for a dense transformer block (B=2, T=2048, E=2048,
H=16, DH=128, FFN 4E) on 8 NeuronCores.

Sharding: 8-way tensor/sequence hybrid.
  - Attention: 2 heads per core, both batches (head-parallel, balanced causal
    work). Each core computes rmsnorm+QKV for all 4096 tokens for its 2 heads.
  - One 8-core mesh AllToAll (4MB/rank) converts head-sharding -> token
    sharding of the attention output.
  - Out-proj, second rmsnorm and the whole FFN are computed fully locally for
    the core's own 512-token chunk (full weights streamed).
All matmuls run in float32r (full-rate fp32, ~1.6e-4 component error).
"""

import math

import numpy as np

N_CORES = 8
B, T, E, H, DH = 2, 2048, 2048, 16, 128
HPC = 2                      # heads per core
DHC = HPC * DH               # 256
FF = 4 * E                   # 8192
TG = B * T                   # 4096 global tokens
CH = TG // N_CORES           # 512 tokens per core chunk
NE = E // 128                # 16 e-tiles
NHT = FF // 128              # 64 hidden tiles
EPS = 1e-8
ISQ = 1.0 / math.sqrt(DH)

_COMPILED = None             # (nc, meta) cache


def _build_nc():
    import concourse.mybir as mybir
    import concourse.tile as tile
    from concourse import bacc

    f32 = mybir.dt.float32
    f32r = mybir.dt.float32r
    AF = mybir.ActivationFunctionType
    ALU = mybir.AluOpType

    nc = bacc.Bacc("TRN2", target_bir_lowering=False, debug=False,
                   num_devices=N_CORES)

    # ---- I/O -------------------------------------------------------------
    x_d = nc.dram_tensor("x", [TG, E], f32, kind="ExternalInput")
    xpbo_d = nc.dram_tensor("xpbo", [CH, E], f32, kind="ExternalInput")
    wq_d = nc.dram_tensor("wq", [128, NE, DHC], f32r, kind="ExternalInput")
    wk_d = nc.dram_tensor("wk", [128, NE, DHC], f32r, kind="ExternalInput")
    wv_d = nc.dram_tensor("wv", [128, NE, DHC], f32r, kind="ExternalInput")
    bqk_d = nc.dram_tensor("bqk", [128, 4], f32, kind="ExternalInput")
    bvrow_d = nc.dram_tensor("bvrow", [1, DHC], f32, kind="ExternalInput")
    maskM_d = nc.dram_tensor("maskM", [128, 1024], f32, kind="ExternalInput")
    ident_d = nc.dram_tensor("ident", [128, 128], f32, kind="ExternalInput")
    onesc_d = nc.dram_tensor("onesc", [128, 1], f32r, kind="ExternalInput")
    onesr_d = nc.dram_tensor("onesr", [1, 128], f32r, kind="ExternalInput")
    wo_d = nc.dram_tensor("wo", [E, E], f32r, kind="ExternalInput")
    b1r_d = nc.dram_tensor("b1r", [128, NHT], f32, kind="ExternalInput")
    w1r_d = nc.dram_tensor("w1r", [NHT, 128, E], f32r, kind="ExternalInput")
    w2_d = nc.dram_tensor("w2", [FF, E], f32r, kind="ExternalInput")
    b2row_d = nc.dram_tensor("b2row", [1, E], f32, kind="ExternalInput")
    out_d = nc.dram_tensor("out", [CH, E], f32, kind="ExternalOutput")

    send_d = nc.dram_tensor("a2a_send", [E, CH], f32r)
    recv_d = nc.dram_tensor("a2a_recv", [E, CH], f32r)

    import concourse.bass as bass

    def bcast_row(row_ap, parts):
        # DMA-broadcast a (1, N) DRAM row across `parts` partitions.
        return bass.AP(tensor=row_ap.tensor, offset=row_ap.offset,
                       ap=[[0, parts]] + list(row_ap.ap)[1:])

    with tile.TileContext(nc) as tc:
        with tc.tile_pool(name="setup", bufs=1) as su:
            ident_s = su.tile([128, 128], f32)
            nc.sync.dma_start(ident_s[:], ident_d.ap()[:, :])
            maskM_s = su.tile([128, 1024], f32)
            nc.sync.dma_start(maskM_s[:], maskM_d.ap()[:, :])
            bqk_s = su.tile([128, 4], f32)
            nc.sync.dma_start(bqk_s[:], bqk_d.ap()[:, :])
            bvbc_s = su.tile([128, DHC], f32)
            nc.sync.dma_start(bvbc_s[:], bcast_row(bvrow_d.ap()[:, :], 128))
            b1r_s = su.tile([128, NHT], f32)
            nc.sync.dma_start(b1r_s[:], b1r_d.ap()[:, :])
            onesc_s = su.tile([128, 1], f32r)
            nc.sync.dma_start(onesc_s[:], onesc_d.ap()[:, :])
            onesr_s = su.tile([1, 128], f32r)
            nc.sync.dma_start(onesr_s[:], onesr_d.ap()[:, :])
            eps_s = su.tile([128, 1], f32)
            nc.vector.memset(eps_s[:], EPS)

            # ============ PHASE 1: rmsnorm + QKV + attention ==============
            with tc.tile_pool(name="pw", bufs=1) as pw, \
                 tc.tile_pool(name="px", bufs=2) as px, \
                 tc.tile_pool(name="pht", bufs=16) as pht, \
                 tc.tile_pool(name="pkT", bufs=4) as pkT, \
                 tc.tile_pool(name="pv", bufs=18) as pv, \
                 tc.tile_pool(name="pqT", bufs=4) as pqT, \
                 tc.tile_pool(name="pprob", bufs=4) as pprob, \
                 tc.tile_pool(name="pstat", bufs=6) as pstat, \
                 tc.tile_pool(name="pz", bufs=2) as pz, \
                 tc.tile_pool(name="prb", bufs=2) as prb, \
                 tc.tile_pool(name="poT", bufs=2) as poT, \
                 tc.tile_pool(name="psum1", bufs=1, space="PSUM") as ps1:

                wq_s = pw.tile([128, NE, DHC], f32r, tag="wq")
                nc.sync.dma_start(wq_s[:], wq_d.ap()[:, :, :])
                wk_s = pw.tile([128, NE, DHC], f32r, tag="wk")
                nc.sync.dma_start(wk_s[:], wk_d.ap()[:, :, :])
                wv_s = pw.tile([128, NE, DHC], f32r, tag="wv")
                nc.sync.dma_start(wv_s[:], wv_d.ap()[:, :, :])

                BSF = nc.vector.BN_STATS_FMAX          # 512
                BSD = nc.vector.BN_STATS_DIM
                BAD = nc.vector.BN_AGGR_DIM
                nsub = E // BSF

                kT_tiles = {}
                v_tiles = {}
                for gc in range(N_CORES):
                    b, jc = divmod(gc, 4)
                    if jc == 0:
                        for h in range(HPC):
                            kT_tiles[(b, h)] = pkT.tile([128, T], f32r, tag="kT",
                                                        name=f"kT_{b}_{h}")
                    # ---- rmsnorm + transpose for this 512-token chunk ----
                    hts = [pht.tile([128, CH], f32r, tag="hT", name=f"hT_{gc}_{i}")
                           for i in range(NE)]
                    for tt in range(4):
                        r0 = gc * CH + tt * 128
                        x_t = px.tile([128, E], f32, tag="x")
                        nc.sync.dma_start(x_t[:], x_d.ap()[r0:r0 + 128, :])
                        stats = pstat.tile([128, nsub, BSD], f32, tag="st")
                        for s in range(nsub):
                            nc.vector.bn_stats(
                                out=stats[:, s, :],
                                in_=x_t[:, s * BSF:(s + 1) * BSF])
                        mv = pstat.tile([128, BAD], f32, tag="mv")
                        nc.vector.bn_aggr(out=mv[:], in_=stats[:])
                        m2 = pstat.tile([128, 1], f32, tag="m2")
                        nc.vector.tensor_mul(m2[:], mv[:, 0:1], mv[:, 0:1])
                        ssum = pstat.tile([128, 1], f32, tag="ss")
                        nc.vector.tensor_add(ssum[:], mv[:, 1:2], m2[:])
                        rms = pstat.tile([128, 1], f32, tag="rms")
                        nc.scalar.activation(rms[:], ssum[:], AF.Sqrt,
                                             bias=eps_s[:], scale=1.0)
                        rinv = pstat.tile([128, 1], f32, tag="rinv")
                        nc.vector.reciprocal(rinv[:], rms[:])
                        h_t = x_t
                        nc.vector.tensor_scalar_mul(h_t[:], x_t[:], rinv[:])
                        for et in range(NE):
                            tp = ps1.tile([128, 128], f32, tag="tp", bufs=2)
                            nc.tensor.transpose(
                                tp[:], h_t[:, et * 128:(et + 1) * 128],
                                ident_s[:])
                            dst = hts[et][:, tt * 128:(tt + 1) * 128]
                            if et % 2 == 0:
                                nc.scalar.activation(dst, tp[:], AF.Copy)
                            else:
                                nc.vector.tensor_copy(dst, tp[:])
                    # ---- QK (one sub-pass per head: 2 psum banks) --------
                    qT_t = []
                    for h in range(HPC):
                        pq = ps1.tile([128, CH], f32, tag="acc", bufs=2, name=f"pq_{gc}_{h}")
                        pk = ps1.tile([128, CH], f32, tag="acc", bufs=2, name=f"pk_{gc}_{h}")
                        for et in range(NE):
                            wqs = wq_s[:, et, h * 128:(h + 1) * 128]
                            wks = wk_s[:, et, h * 128:(h + 1) * 128]
                            nc.tensor.matmul(pq[:], wqs, hts[et][:],
                                             start=(et == 0),
                                             stop=(et == NE - 1))
                            nc.tensor.matmul(pk[:], wks, hts[et][:],
                                             start=(et == 0),
                                             stop=(et == NE - 1))
                        qt = pqT.tile([128, CH], f32r, tag="qT")
                        nc.scalar.activation(qt[:], pq[:], AF.Identity,
                                             bias=bqk_s[:, h:h + 1])
                        qT_t.append(qt)
                        nc.scalar.activation(
                            kT_tiles[(b, h)][:, jc * CH:(jc + 1) * CH],
                            pk[:], AF.Identity, bias=bqk_s[:, 2 + h:3 + h])
                    # ---- V (two sub-passes of 2 t-tiles) -----------------
                    for sub in range(2):
                        pvs = [ps1.tile([128, DHC], f32, tag="acc", bufs=2, name=f"pv_{gc}_{sub}_{i}")
                               for i in range(2)]
                        for et in range(NE):
                            for i in range(2):
                                tt = sub * 2 + i
                                nc.tensor.matmul(
                                    pvs[i][:],
                                    hts[et][:, tt * 128:(tt + 1) * 128],
                                    wv_s[:, et, :],
                                    start=(et == 0), stop=(et == NE - 1))
                        for i in range(2):
                            tt = sub * 2 + i
                            vt = pv.tile([128, DHC], f32r, tag="v")
                            v_tiles[(b, jc * 4 + tt)] = vt
                            nc.vector.tensor_tensor(
                                out=vt[:], in0=pvs[i][:], in1=bvbc_s[:],
                                op=ALU.add)
                    # ---- attention for this q-chunk ----------------------
                    nkt = (jc + 1) * 4
                    for h in range(HPC):
                        zp = ps1.tile([1, CH], f32, tag="z", bufs=1)
                        op_ = ps1.tile([128, CH], f32, tag="o", bufs=1)
                        for i in range(nkt):
                            sp = ps1.tile([128, CH], f32, tag="sT", bufs=2)
                            nc.tensor.matmul(
                                sp[:],
                                kT_tiles[(b, h)][:, i * 128:(i + 1) * 128],
                                qT_t[h][:], start=True, stop=True)
                            p_t = pprob.tile([128, CH], f32r, tag="p")
                            nc.scalar.activation(p_t[:], sp[:], AF.Exp,
                                                 scale=ISQ)
                            m = i - jc * 4
                            if m >= 0:
                                msl = maskM_s[:, 384 - 128 * m:
                                              896 - 128 * m]
                                nc.vector.tensor_tensor(
                                    out=p_t[:], in0=p_t[:], in1=msl,
                                    op=ALU.mult)
                            nc.tensor.matmul(zp[:], onesc_s[:], p_t[:],
                                             start=(i == 0),
                                             stop=(i == nkt - 1))
                            nc.tensor.matmul(
                                op_[:],
                                v_tiles[(b, i)][:, h * 128:(h + 1) * 128],
                                p_t[:], start=(i == 0), stop=(i == nkt - 1))
                        zr = pz.tile([1, CH], f32, tag="zr")
                        nc.vector.reciprocal(zr[:], zp[:])
                        zrr = pz.tile([1, CH], f32r, tag="zrr")
                        nc.vector.tensor_copy(zrr[:], zr[:])
                        rbp = ps1.tile([128, CH], f32, tag="sT", bufs=2)
                        nc.tensor.matmul(rbp[:], onesr_s[:], zrr[:],
                                         start=True, stop=True)
                        rb = prb.tile([128, CH], f32, tag="rb")
                        nc.scalar.copy(rb[:], rbp[:])
                        oT = poT.tile([128, CH], f32r, tag="oT")
                        nc.vector.tensor_tensor(out=oT[:], in0=op_[:],
                                                in1=rb[:], op=ALU.mult)
                        r0 = gc * DHC + h * 128
                        nc.sync.dma_start(send_d.ap()[r0:r0 + 128, :], oT[:])

            # ============ PHASE 2: AllToAll ===============================
            import os as _os
            if _os.environ.get("KERNEL_NO_CC"):
                nc.sync.dma_start(recv_d.ap()[:, :], send_d.ap()[:, :])
            else:
                nc.gpsimd.collective_compute(
                    "AllToAll", ALU.bypass,
                    replica_groups=[list(range(N_CORES))],
                    ins=[send_d.ap()[:, :]],
                    outs=[recv_d.ap()[:, :]],
                )

            # ============ PHASE 3+4 =======================================
            with tc.tile_pool(name="pxa", bufs=4) as pxa, \
                 tc.tile_pool(name="ph2T", bufs=16) as ph2T:
                xa = []
                for tt in range(4):
                    t = pxa.tile([128, E], f32, tag="xa", name=f"xa_{tt}")
                    nc.sync.dma_start(t[:],
                                      xpbo_d.ap()[tt * 128:(tt + 1) * 128, :])
                    xa.append(t)

                with tc.tile_pool(name="po", bufs=2) as po, \
                     tc.tile_pool(name="pwo", bufs=2) as pwo, \
                     tc.tile_pool(name="ph2", bufs=1) as ph2, \
                     tc.tile_pool(name="pstat3", bufs=6) as pstat3, \
                     tc.tile_pool(name="psum3", bufs=1, space="PSUM") as ps3:
                    # ---- out-projection for own chunk --------------------
                    for ec in range(4):
                        ops = [ps3.tile([128, CH], f32, tag="op", bufs=4, name=f"ops_{ec}_{i}")
                               for i in range(4)]
                        for ht in range(NE):
                            ot = po.tile([128, CH], f32r, tag="ot")
                            nc.sync.dma_start(
                                ot[:],
                                recv_d.ap()[ht * 128:(ht + 1) * 128, :])
                            wot = pwo.tile([128, CH], f32r, tag="wo")
                            nc.sync.dma_start(
                                wot[:],
                                wo_d.ap()[ht * 128:(ht + 1) * 128,
                                          ec * CH:(ec + 1) * CH])
                            for tt in range(4):
                                nc.tensor.matmul(
                                    ops[tt][:],
                                    ot[:, tt * 128:(tt + 1) * 128], wot[:],
                                    start=(ht == 0), stop=(ht == NE - 1))
                        for tt in range(4):
                            sl = xa[tt][:, ec * CH:(ec + 1) * CH]
                            nc.vector.tensor_tensor(out=sl, in0=ops[tt][:],
                                                    in1=sl, op=ALU.add)
                    # ---- rmsnorm 2 + transpose ---------------------------
                    BSF = nc.vector.BN_STATS_FMAX
                    BSD = nc.vector.BN_STATS_DIM
                    BAD = nc.vector.BN_AGGR_DIM
                    nsub = E // BSF
                    h2T = [ph2T.tile([128, CH], f32r, tag="h2T", name=f"h2T_{i}")
                           for i in range(NE)]
                    for tt in range(4):
                        stats = pstat3.tile([128, nsub, BSD], f32, tag="st")
                        for s in range(nsub):
                            nc.vector.bn_stats(
                                out=stats[:, s, :],
                                in_=xa[tt][:, s * BSF:(s + 1) * BSF])
                        mv = pstat3.tile([128, BAD], f32, tag="mv")
                        nc.vector.bn_aggr(out=mv[:], in_=stats[:])
                        m2 = pstat3.tile([128, 1], f32, tag="m2")
                        nc.vector.tensor_mul(m2[:], mv[:, 0:1], mv[:, 0:1])
                        ssum = pstat3.tile([128, 1], f32, tag="ss")
                        nc.vector.tensor_add(ssum[:], mv[:, 1:2], m2[:])
                        rms = pstat3.tile([128, 1], f32, tag="rms")
                        nc.scalar.activation(rms[:], ssum[:], AF.Sqrt,
                                             bias=eps_s[:], scale=1.0)
                        rinv = pstat3.tile([128, 1], f32, tag="rinv")
                        nc.vector.reciprocal(rinv[:], rms[:])
                        h2_t = ph2.tile([128, E], f32, tag="h2")
                        nc.vector.tensor_scalar_mul(h2_t[:], xa[tt][:],
                                                    rinv[:])
                        for et in range(NE):
                            tp = ps3.tile([128, 128], f32, tag="tp", bufs=2)
                            nc.tensor.transpose(
                                tp[:], h2_t[:, et * 128:(et + 1) * 128],
                                ident_s[:])
                            dst = h2T[et][:, tt * 128:(tt + 1) * 128]
                            if et % 2 == 0:
                                nc.scalar.activation(dst, tp[:], AF.Copy)
                            else:
                                nc.vector.tensor_copy(dst, tp[:])

                # ---- FFN --------------------------------------------------
                with tc.tile_pool(name="paT", bufs=16) as paT, \
                     tc.tile_pool(name="pacc", bufs=4) as pacc, \
                     tc.tile_pool(name="pw1", bufs=2) as pw1, \
                     tc.tile_pool(name="pw2", bufs=4) as pw2, \
                     tc.tile_pool(name="psum4", bufs=1, space="PSUM") as ps4:
                    b2bc_s = pacc.tile([128, E], f32, tag="b2bc")
                    nc.sync.dma_start(b2bc_s[:],
                                      bcast_row(b2row_d.ap()[:, :], 128))
                    acc = [pacc.tile([128, E], f32, tag="acc", name=f"acc_{i}")
                           for i in range(4)]
                    for grp in range(4):
                        aTs = []
                        for htl in range(16):
                            ht = grp * 16 + htl
                            w1t = pw1.tile([128, E], f32r, tag="w1")
                            nc.sync.dma_start(w1t[:], w1r_d.ap()[ht, :, :])
                            f1 = ps4.tile([128, CH], f32, tag="f1", bufs=3)
                            for et in range(NE):
                                nc.tensor.matmul(
                                    f1[:], w1t[:, et * 128:(et + 1) * 128],
                                    h2T[et][:], start=(et == 0),
                                    stop=(et == NE - 1))
                            aT = paT.tile([128, CH], f32r, tag="aT")
                            aTs.append(aT)
                            nc.scalar.activation(aT[:], f1[:], AF.Gelu,
                                                 bias=b1r_s[:, ht:ht + 1])
                        for ec in range(4):
                            f2s = [ps4.tile([128, CH], f32, tag="f2", bufs=4, name=f"f2_{grp}_{ec}_{i}")
                                   for i in range(4)]
                            for htl in range(16):
                                ht = grp * 16 + htl
                                w2t = pw2.tile([128, CH], f32r, tag="w2")
                                nc.sync.dma_start(
                                    w2t[:],
                                    w2_d.ap()[ht * 128:(ht + 1) * 128,
                                              ec * CH:(ec + 1) * CH])
                                for tt in range(4):
                                    nc.tensor.matmul(
                                        f2s[tt][:],
                                        aTs[htl][:, tt * 128:(tt + 1) * 128],
                                        w2t[:], start=(htl == 0),
                                        stop=(htl == 15))
                            for tt in range(4):
                                sl = acc[tt][:, ec * CH:(ec + 1) * CH]
                                if grp == 0:
                                    nc.vector.tensor_copy(sl, f2s[tt][:])
                                else:
                                    nc.vector.tensor_tensor(
                                        out=sl, in0=f2s[tt][:], in1=sl,
                                        op=ALU.add)
                    for tt in range(4):
                        nc.vector.tensor_tensor(out=acc[tt][:],
                                                in0=xa[tt][:],
                                                in1=acc[tt][:], op=ALU.add)
                        nc.vector.tensor_tensor(out=acc[tt][:],
                                                in0=b2bc_s[:],
                                                in1=acc[tt][:], op=ALU.add)
                        nc.sync.dma_start(
                            out_d.ap()[tt * 128:(tt + 1) * 128, :],
                            acc[tt][:])

    nc.compile()
    return nc


def _host_prep(x, wq, bq, wk, bk, wv, bv, wo, bo, w1, b1, w2, b2, g1, g2):
    """Fold gains into weights, build per-core in_maps."""
    f = np.float32
    x = np.asarray(x, f)
    g1 = np.asarray(g1, f)
    g2 = np.asarray(g2, f)
    wq = np.asarray(wq, f) * g1[None, :, None]
    wk = np.asarray(wk, f) * g1[None, :, None]
    wv = np.asarray(wv, f) * g1[None, :, None]
    w1 = np.asarray(w1, f) * g2[:, None]
    w2 = np.asarray(w2, f)
    wo = np.asarray(wo, f)
    bo = np.asarray(bo, f)

    x_flat = np.ascontiguousarray(x.reshape(TG, E))

    # masks: M[k, j] = 1 iff j >= k + 384
    kk = np.arange(128)[:, None]
    jj = np.arange(1024)[None, :]
    maskM = (jj >= kk + 384).astype(f)
    ident = np.eye(128, dtype=f)
    onesc = np.ones((128, 1), f)
    onesr = np.ones((1, 128), f)

    # w1r[ht, p, et*128+m] = w1[et*128+p, ht*128+m]
    w1r = np.ascontiguousarray(
        w1.reshape(NE, 128, NHT, 128).transpose(2, 1, 0, 3).reshape(
            NHT, 128, E))
    b1r = np.ascontiguousarray(np.asarray(b1, f).reshape(NHT, 128).T)
    b2row = np.asarray(b2, f).reshape(1, E)

    in_maps = []
    for c in range(N_CORES):
        h0 = HPC * c
        # (E, DHC) with column h*128+d  ->  (128, NE, DHC) [p, et, m]
        wq_c = np.ascontiguousarray(
            wq[h0:h0 + HPC].transpose(1, 0, 2).reshape(E, DHC)
            .reshape(NE, 128, DHC).transpose(1, 0, 2))
        wk_c = np.ascontiguousarray(
            wk[h0:h0 + HPC].transpose(1, 0, 2).reshape(E, DHC)
            .reshape(NE, 128, DHC).transpose(1, 0, 2))
        wv_c = np.ascontiguousarray(
            wv[h0:h0 + HPC].transpose(1, 0, 2).reshape(E, DHC)
            .reshape(NE, 128, DHC).transpose(1, 0, 2))
        bqk = np.stack([np.asarray(bq, f)[h0], np.asarray(bq, f)[h0 + 1],
                        np.asarray(bk, f)[h0], np.asarray(bk, f)[h0 + 1]],
                       axis=1)                        # (128, 4)
        bvrow = np.asarray(bv, f)[h0:h0 + HPC].reshape(1, DHC)
        xpbo = x_flat[c * CH:(c + 1) * CH] + bo[None, :]
        in_maps.append({
            "x": x_flat,
            "xpbo": np.ascontiguousarray(xpbo),
            "wq": wq_c, "wk": wk_c, "wv": wv_c,
            "bqk": np.ascontiguousarray(bqk),
            "bvrow": np.ascontiguousarray(bvrow),
            "maskM": maskM, "ident": ident,
            "onesc": onesc, "onesr": onesr,
            "wo": wo, "b1r": b1r, "w1r": w1r, "w2": w2,
            "b2row": b2row,
        })
    return in_maps


def _get_compiled():
    global _COMPILED
    if _COMPILED is None:
        _COMPILED = _build_nc()
    return _COMPILED


def kernel(**inputs) -> np.ndarray:
    from concourse.bass_utils import run_bass_kernel_spmd

    nc = _get_compiled()
    in_maps = _host_prep(**inputs)
    res = run_bass_kernel_spmd(nc, in_maps, list(range(N_CORES))).results
    out = np.empty((TG, E), np.float32)
    for c in range(N_CORES):
        out[c * CH:(c + 1) * CH] = res[c]["out"]
    return out.reshape(B, T, E)
